# revision 10
# baseline (speedup 1.0000x reference)
"""3x3 valid conv (single channel) on 8 TRN2 NeuronCores.

Strategy: shard X row-wise (512 output rows/core), fp16 end-to-end.
The problem is memory-bound at fp32 (34MB/core); converting X to fp16 on
host and storing y as fp16 (upcast on host) halves HBM traffic to
~17MB/core. fp16 matmul runs at 1 row/cycle with exact f32 PSUM
accumulation, so the only precision cost is the input/output rounding:
~8e-4 relative -- far inside the 2e-2 gate. At fp16 the PE becomes the
critical engine (~45us of moving rows), so the schedule optimizes PE
stream continuity.

Per core, five row strips: four full strips load 128 input rows each
(rows 126s..126s+127, re-reading the 2-row halo from HBM) and produce
126 output rows via 3 banded matmuls per 512-col PSUM tile; a mini-strip
loads rows 504..513 (10 rows, im2col-skewed so one K=30 matmul per col
tile) produces the remaining 8 output rows, its 16 col tiles interleaved
into strip 3's so it adds no tail.

Startup: the first matmul needs the band + the first X chunk; each DMA
ring has ~2.5us of cold latency, so the band rides sync (earliest ring)
and the first two X chunks ride the vector/scalar rings in parallel.
Meanwhile the PE ramps its p-state (0.65->2.4GHz) on dummy matmuls over
a memset tile so the real stream starts at speed. PSUM drains (bias add
+ f32->fp16 cast) alternate vector/scalar so a single engine's drain
rate can't gate the PE.
"""

import sys

sys.path.insert(0, "/opt/trn_rl_repo")

import numpy as np
from concourse import bass, mybir
from concourse.bass_utils import run_bass_kernel_spmd
from concourse.tile import TileContext

F32 = mybir.dt.float32
F16 = mybir.dt.float16

H, WIDTH = 4096, 8192
KH, KW = 3, 3
OH, OW = H - KH + 1, WIDTH - KW + 1
N_CORES = 8
RPC = H // N_CORES          # 512 output rows produced per core
IN_ROWS = RPC + KH - 1      # 514 input rows per core (2-row halo)
N_COL_TILES = 16            # 15 x 512 + 1 x 510 = 8190


def _split_multi_waits(nc, max_waits=1):
    # This container's walrus rejects >1 sync-wait command per instruction
    # (CoreV3 setupSyncWait). Tile attaches one wait per producing logical
    # processor to a single instruction; hoist the excess onto same-engine
    # Drain carriers inserted immediately before it.
    for fn in nc.m.functions:
        for bb in fn.blocks:
            out = []
            changed = False
            for inst in bb.instructions:
                si = inst.sync_info
                waits = list(si.on_wait) if si and si.on_wait else []
                if len(waits) > max_waits:
                    rest = waits[max_waits:]
                    for j in range(0, len(rest), max_waits):
                        carrier = mybir.InstDrain(
                            name=nc.get_next_instruction_name(), ins=[], outs=[]
                        )
                        carrier.engine = inst.engine
                        carrier.sync_info = mybir.SyncInfo(
                            on_wait=rest[j : j + max_waits], on_update=[]
                        )
                        out.append(carrier)
                    si.on_wait = waits[:max_waits]
                    changed = True
                out.append(inst)
            if changed:
                bb.instructions = out


def _build(split_waits=True):
    nc = bass.Bass()
    x = nc.declare_dram_parameter("x", [IN_ROWS, WIDTH], F16, isOutput=False)
    bands = nc.declare_dram_parameter("bands", [128, 3 * 128], F16, isOutput=False)
    bands4 = nc.declare_dram_parameter("bands4", [32, 8], F16, isOutput=False)
    bias = nc.declare_dram_parameter("bias", [128, 1], F32, isOutput=False)
    y = nc.declare_dram_parameter("y", [RPC, OW], F16, isOutput=True)

    ident = mybir.ActivationFunctionType.Identity

    with TileContext(nc) as tc:
        with (
            tc.tile_pool(name="const", bufs=1) as cpool,
            tc.tile_pool(name="xin", bufs=3) as xpool,
            tc.tile_pool(name="stage", bufs=3) as spool,
            tc.tile_pool(name="psum", bufs=8, space="PSUM") as ppool,
        ):
            # --- PE p-state warmup: dummy matmuls over a memset tile.
            # No DMA dependency, so they start right after the preamble
            # barrier and the clock is ramping while the first X chunk is
            # still in flight.
            warm_t = cpool.tile([128, 512], F16)
            nc.gpsimd.memset(warm_t[:], 0.0)
            ps_w = ppool.tile([128, 512], F32, tag="ps")
            for _ in range(4):
                nc.tensor.matmul(
                    ps_w[:126, :512],
                    warm_t[:, 0:126],
                    warm_t[:, 0:512],
                    start=True,
                    stop=True,
                )

            # --- constants and first strip chunks, ordered so each of the
            # three cold DMA rings starts its first transfer immediately:
            # sync gets the band (the first matmul's LDWEIGHTS needs it),
            # gpsimd and scalar each get one early X chunk. The tiny bias
            # and band4 queue behind gpsimd's chunk.
            band_f = cpool.tile([128, 3 * 128], F16)
            nc.sync.dma_start(out=band_f[:], in_=bands[:])
            xt0 = xpool.tile([128, WIDTH], F16, tag="xt")
            nc.gpsimd.dma_start(out=xt0[:, 0:528], in_=x[0:128, 0:528])
            nc.scalar.dma_start(out=xt0[:, 528:1040], in_=x[0:128, 528:1040])
            for a, b in [(1040, 1552), (1552, 2576), (2576, 4624), (4624, 8192)]:
                nc.sync.dma_start(out=xt0[:, a:b], in_=x[0:128, a:b])
            bias_t = cpool.tile([128, 1], F32)
            nc.gpsimd.dma_start(out=bias_t[:], in_=bias[:])
            band4_f = cpool.tile([32, 8], F16)
            nc.gpsimd.dma_start(out=band4_f[:], in_=bands4[:])
            stage4 = cpool.tile([8, WIDTH], F16)
            # prime the ACT function table (1.3us, once) before the first
            # scalar drain needs it
            prime_t = cpool.tile([128, 1], F32)
            nc.scalar.activation(prime_t[:1, :], bias_t[:1, :], ident,
                                 bias=bias_t[:1, :], scale=1.0)

            def drain(ct, dst, src, npart):
                # alternate engines so neither gates the PE
                if ct % 2 == 0:
                    nc.scalar.activation(dst, src, ident,
                                         bias=bias_t[:npart, :], scale=1.0)
                else:
                    nc.vector.tensor_scalar_add(dst, src, bias_t[:npart, :])

            # scalar store-issues are LAGGED by one group: emitted only
            # after the next group's drains, so their drain-sems are already
            # satisfied and the scalar queue (which also carries half the
            # drains) never head-of-line blocks
            pending = []

            def flush_pending():
                while pending:
                    dst, srcap = pending.pop(0)
                    nc.scalar.dma_start(out=dst, in_=srcap)

            def mini_tile(ct, x4):
                # one col tile of the 8-row mini-strip (im2col, K=30)
                c0 = ct * 512
                n = 512 if ct < N_COL_TILES - 1 else 510
                ps = ppool.tile([128, 512], F32, tag="ps")
                nc.tensor.matmul(
                    ps[:8, :n],
                    band4_f[0:30, 0:8],
                    x4[0:30, c0 : c0 + n],
                    start=True,
                    stop=True,
                )
                drain(ct + 1, stage4[:8, c0 : c0 + n], ps[:8, :n], 8)
                if ct % 4 == 3:
                    q0 = (ct - 3) * 512
                    q1 = min(ct * 512 + n, OW)
                    nc.sync.dma_start(
                        out=y[504:512, q0:q1], in_=stage4[:8, q0:q1]
                    )

            def full_strip(s, xt, last, x4=None):
                r0 = 126 * s
                for g in range(2):
                    stage = spool.tile([128, 4096], F16, tag="st")
                    for j in range(8):
                        ct = g * 8 + j
                        c0 = ct * 512
                        n = 512 if ct < N_COL_TILES - 1 else 510
                        ps = ppool.tile([128, 512], F32, tag="ps")
                        for dj in range(KW):
                            nc.tensor.matmul(
                                ps[:126, :n],
                                band_f[:, dj * 128 : dj * 128 + 126],
                                xt[:, c0 + dj : c0 + dj + n],
                                start=(dj == 0),
                                stop=(dj == KW - 1),
                            )
                        drain(ct, stage[:126, j * 512 : j * 512 + n],
                              ps[:126, :n], 126)
                        if x4 is not None:
                            # interleave the mini-strip's col tiles so they
                            # add no tail after strip 3
                            mini_tile(ct, x4)
                        if last and g == 1 and j in (3, 5, 7):
                            # drip out the final stage in pieces so the very
                            # last store is small
                            lo = {3: 0, 5: 2048, 7: 3072}[j]
                            hi = {3: 2048, 5: 3072, 7: 4094}[j]
                            nc.scalar.dma_start(
                                out=y[r0 : r0 + 126, 4096 + lo : 4096 + hi],
                                in_=stage[:126, lo:hi],
                            )
                    flush_pending()
                    if not (last and g == 1):
                        gw = 4096 if g == 0 else 4094
                        if last and g == 0:
                            # strip 3's g0 store rides the by-then-quiet SP
                            # ring, immediately (nothing queues behind it)
                            nc.sync.dma_start(
                                out=y[r0 : r0 + 126, 0:gw],
                                in_=stage[:126, :gw],
                            )
                        else:
                            pending.append((
                                y[r0 : r0 + 126, g * 4096 : g * 4096 + gw],
                                stage[:126, :gw],
                            ))

            for s in range(3):
                r0 = 126 * s
                if s == 0:
                    xt = xt0
                else:
                    xt = xpool.tile([128, WIDTH], F16, tag="xt")
                    chunks = [(0, 2048), (2048, 8192)] if s == 1 else [(0, 4096), (4096, 8192)]
                    for a, b in chunks:
                        nc.sync.dma_start(out=xt[:, a:b], in_=x[r0 : r0 + 128, a:b])
                full_strip(s, xt, last=False)

            # strip-3 loads first: xt3 reuses s0's buffer (free earliest)
            # so its 2MB lands early. The mini-strip tile follows (s1's
            # buffer); its im2col layout -- partition 3r + dj holds
            # X[504+r, dj:] -- means one K=30 matmul per col tile instead
            # of three K=10 ones, and the dj-interleaved partitions spread
            # each 10-row load over 8 SBUF ports instead of 2-3.
            xt3 = xpool.tile([128, WIDTH], F16, tag="xt")
            for a, b in [(0, 4096), (4096, 8192)]:
                nc.sync.dma_start(out=xt3[:, a:b], in_=x[378 : 378 + 128, a:b])
            x4 = xpool.tile([128, WIDTH], F16, tag="xt")
            for dj in range(KW):
                nc.sync.dma_start(
                    out=x4[dj : 28 + dj + 1 : 3, 0 : WIDTH - dj],
                    in_=x[504:514, dj:WIDTH],
                )

            full_strip(3, xt3, last=True, x4=x4)

    if split_waits:
        _split_multi_waits(nc)
    return nc


_NC_CACHE = None


def _get_nc():
    global _NC_CACHE
    if _NC_CACHE is None:
        _NC_CACHE = _build()
    return _NC_CACHE


def _make_host_inputs(X, W, b):
    Xh = np.ascontiguousarray(np.asarray(X, dtype=np.float32).astype(np.float16))
    W = np.asarray(W, dtype=np.float32)
    b = np.asarray(b, dtype=np.float32)

    bands = np.zeros((128, 3 * 128), dtype=np.float16)
    mm = np.arange(126)
    for dj in range(KW):
        for dk in range(KH):
            # B_dj[m+dk, m] = W[dk, dj] for every output row m
            bands[mm + dk, dj * 128 + mm] = W[dk, dj]
    # mini-strip im2col band: partition 3r + dj = input local row 504+r
    # shifted by dj cols; col m = output local row 504+m; B4[3r+dj, m] =
    # W[r-m, dj]
    bands4 = np.zeros((32, 8), dtype=np.float16)
    m8 = np.arange(8)
    for dj in range(KW):
        for dk in range(KH):
            bands4[3 * (m8 + dk) + dj, m8] = W[dk, dj]
    bias = np.full((128, 1), float(b[0]), dtype=np.float32)

    in_maps = []
    for i in range(N_CORES):
        r0 = i * RPC
        avail = min(IN_ROWS, H - r0)
        if avail == IN_ROWS:
            shard = Xh[r0 : r0 + IN_ROWS]
        else:
            shard = np.zeros((IN_ROWS, WIDTH), dtype=np.float16)
            shard[:avail] = Xh[r0 : r0 + avail]
        in_maps.append({"x": shard, "bands": bands, "bands4": bands4, "bias": bias})
    return in_maps


def _assemble(results):
    out = np.empty((OH, OW), dtype=np.float32)
    for i in range(N_CORES):
        r0 = i * RPC
        take = min(RPC, OH - r0)
        out[r0 : r0 + take] = results[i]["y"][:take].astype(np.float32)
    return out


def run(X, W, b, trace=False):
    nc = _get_nc()
    in_maps = _make_host_inputs(X, W, b)
    res = run_bass_kernel_spmd(nc, in_maps, list(range(N_CORES)), trace=trace)
    return _assemble(res.results), res


def kernel(X, W, b):
    out, _ = run(X, W, b)
    return out


# revision 11
# speedup vs baseline: 1.1647x; 1.1647x over previous
"""3x3 valid conv (single channel) on 8 TRN2 NeuronCores.

Strategy: shard X row-wise (512 output rows/core), fp16 end-to-end.
The problem is memory-bound at fp32 (34MB/core); converting X to fp16 on
host and storing y as fp16 (upcast on host) halves HBM traffic to
~17MB/core. fp16 matmul runs at 1 row/cycle with exact f32 PSUM
accumulation, so the only precision cost is the input/output rounding:
~8e-4 relative -- far inside the 2e-2 gate. At fp16 the PE becomes the
critical engine (~45us of moving rows), so the schedule optimizes PE
stream continuity.

Per core, five row strips: four full strips load 128 input rows each
(rows 126s..126s+127, re-reading the 2-row halo from HBM) and produce
126 output rows via 3 banded matmuls per 512-col PSUM tile; a mini-strip
loads rows 504..513 (10 rows, im2col-skewed so one K=30 matmul per col
tile) produces the remaining 8 output rows, its 16 col tiles interleaved
into strip 3's so it adds no tail.

Startup: the first matmul needs the band + the first X chunk; each DMA
ring has ~2.5us of cold latency, so the band rides sync (earliest ring)
and the first two X chunks ride the vector/scalar rings in parallel.
Meanwhile the PE ramps its p-state (0.65->2.4GHz) on dummy matmuls over
a memset tile so the real stream starts at speed. PSUM drains (bias add
+ f32->fp16 cast) alternate vector/scalar so a single engine's drain
rate can't gate the PE.
"""

import sys

sys.path.insert(0, "/opt/trn_rl_repo")

import numpy as np
from concourse import bass, mybir
from concourse.bass_utils import run_bass_kernel_spmd
from concourse.tile import TileContext

F32 = mybir.dt.float32
F16 = mybir.dt.float16

H, WIDTH = 4096, 8192
KH, KW = 3, 3
OH, OW = H - KH + 1, WIDTH - KW + 1
N_CORES = 8
RPC = H // N_CORES          # 512 output rows produced per core
IN_ROWS = RPC + KH - 1      # 514 input rows per core (2-row halo)
N_COL_TILES = 16            # 15 x 512 + 1 x 510 = 8190


def _split_multi_waits(nc, max_waits=1):
    # This container's walrus rejects >1 sync-wait command per instruction
    # (CoreV3 setupSyncWait). Tile attaches one wait per producing logical
    # processor to a single instruction; hoist the excess onto same-engine
    # Drain carriers inserted immediately before it.
    for fn in nc.m.functions:
        for bb in fn.blocks:
            out = []
            changed = False
            for inst in bb.instructions:
                si = inst.sync_info
                waits = list(si.on_wait) if si and si.on_wait else []
                if len(waits) > max_waits:
                    rest = waits[max_waits:]
                    for j in range(0, len(rest), max_waits):
                        carrier = mybir.InstDrain(
                            name=nc.get_next_instruction_name(), ins=[], outs=[]
                        )
                        carrier.engine = inst.engine
                        carrier.sync_info = mybir.SyncInfo(
                            on_wait=rest[j : j + max_waits], on_update=[]
                        )
                        out.append(carrier)
                    si.on_wait = waits[:max_waits]
                    changed = True
                out.append(inst)
            if changed:
                bb.instructions = out


def _build(split_waits=True):
    nc = bass.Bass()
    x = nc.declare_dram_parameter("x", [IN_ROWS, WIDTH], F16, isOutput=False)
    bands = nc.declare_dram_parameter("bands", [128, 3 * 128], F16, isOutput=False)
    bands4 = nc.declare_dram_parameter("bands4", [32, 8], F16, isOutput=False)
    bias = nc.declare_dram_parameter("bias", [128, 1], F32, isOutput=False)
    y = nc.declare_dram_parameter("y", [RPC, OW], F16, isOutput=True)

    ident = mybir.ActivationFunctionType.Identity

    with TileContext(nc) as tc:
        with (
            tc.tile_pool(name="const", bufs=1) as cpool,
            tc.tile_pool(name="xin", bufs=3) as xpool,
            tc.tile_pool(name="stage", bufs=3) as spool,
            tc.tile_pool(name="psum", bufs=8, space="PSUM") as ppool,
        ):
            # --- PE p-state warmup: dummy matmuls over a memset tile.
            # No DMA dependency, so they start right after the preamble
            # barrier and the clock is ramping while the first X chunk is
            # still in flight.
            warm_t = cpool.tile([128, 512], F16)
            nc.gpsimd.memset(warm_t[:], 0.0)
            ps_w = ppool.tile([128, 512], F32, tag="ps")
            for _ in range(4):
                nc.tensor.matmul(
                    ps_w[:126, :512],
                    warm_t[:, 0:126],
                    warm_t[:, 0:512],
                    start=True,
                    stop=True,
                )

            # --- constants and first strip chunks, ordered so each of the
            # three cold DMA rings starts its first transfer immediately:
            # sync gets the band (the first matmul's LDWEIGHTS needs it),
            # gpsimd and scalar each get one early X chunk. The tiny bias
            # and band4 queue behind gpsimd's chunk.
            band_f = cpool.tile([128, 3 * 128], F16)
            nc.sync.dma_start(out=band_f[:], in_=bands[:])
            xt0 = xpool.tile([128, WIDTH], F16, tag="xt")
            nc.gpsimd.dma_start(out=xt0[:, 0:528], in_=x[0:128, 0:528])
            nc.scalar.dma_start(out=xt0[:, 528:1040], in_=x[0:128, 528:1040])
            for a, b in [(1040, 1552), (1552, 2576), (2576, 4624), (4624, 8192)]:
                nc.sync.dma_start(out=xt0[:, a:b], in_=x[0:128, a:b])
            bias_t = cpool.tile([128, 1], F32)
            nc.gpsimd.dma_start(out=bias_t[:], in_=bias[:])
            band4_f = cpool.tile([32, 8], F16)
            nc.gpsimd.dma_start(out=band4_f[:], in_=bands4[:])
            stage4 = cpool.tile([8, WIDTH], F16)
            # prime the ACT function table (1.3us, once) before the first
            # scalar drain needs it
            prime_t = cpool.tile([128, 1], F32)
            nc.scalar.activation(prime_t[:1, :], bias_t[:1, :], ident,
                                 bias=bias_t[:1, :], scale=1.0)

            def drain(ct, dst, src, npart):
                # alternate engines so neither gates the PE
                if ct % 2 == 0:
                    nc.scalar.activation(dst, src, ident,
                                         bias=bias_t[:npart, :], scale=1.0)
                else:
                    nc.vector.tensor_scalar_add(dst, src, bias_t[:npart, :])

            # scalar store-issues are LAGGED by one group: emitted only
            # after the next group's drains, so their drain-sems are already
            # satisfied and the scalar queue (which also carries half the
            # drains) never head-of-line blocks
            pending = []

            def flush_pending():
                while pending:
                    dst, srcap = pending.pop(0)
                    nc.scalar.dma_start(out=dst, in_=srcap)

            def mini_tile(ct, x4):
                # one col tile of the 8-row mini-strip (im2col, K=30)
                c0 = ct * 512
                n = 512 if ct < N_COL_TILES - 1 else 510
                ps = ppool.tile([128, 512], F32, tag="ps")
                nc.tensor.matmul(
                    ps[:8, :n],
                    band4_f[0:30, 0:8],
                    x4[0:30, c0 : c0 + n],
                    start=True,
                    stop=True,
                )
                # mini drains ride the otherwise-idle Pool engine so the
                # vector/scalar drain pair never gates the PE in the
                # interleaved endgame
                nc.gpsimd.tensor_scalar_add(
                    stage4[:8, c0 : c0 + n], ps[:8, :n], bias_t[:8, :]
                )
                if ct % 4 == 3:
                    q0 = (ct - 3) * 512
                    q1 = min(ct * 512 + n, OW)
                    nc.sync.dma_start(
                        out=y[504:512, q0:q1], in_=stage4[:8, q0:q1]
                    )

            def full_strip(s, xt, last, x4=None):
                r0 = 126 * s
                for g in range(2):
                    stage = spool.tile([128, 4096], F16, tag="st")
                    for j in range(8):
                        ct = g * 8 + j
                        c0 = ct * 512
                        n = 512 if ct < N_COL_TILES - 1 else 510
                        ps = ppool.tile([128, 512], F32, tag="ps")
                        for dj in range(KW):
                            nc.tensor.matmul(
                                ps[:126, :n],
                                band_f[:, dj * 128 : dj * 128 + 126],
                                xt[:, c0 + dj : c0 + dj + n],
                                start=(dj == 0),
                                stop=(dj == KW - 1),
                            )
                        drain(ct, stage[:126, j * 512 : j * 512 + n],
                              ps[:126, :n], 126)
                        if x4 is not None:
                            # interleave the mini-strip's col tiles so they
                            # add no tail after strip 3
                            mini_tile(ct, x4)
                        if last and g == 1 and j in (3, 5, 7):
                            # drip out the final stage in pieces so the very
                            # last store is small
                            lo = {3: 0, 5: 2048, 7: 3072}[j]
                            hi = {3: 2048, 5: 3072, 7: 4094}[j]
                            nc.scalar.dma_start(
                                out=y[r0 : r0 + 126, 4096 + lo : 4096 + hi],
                                in_=stage[:126, lo:hi],
                            )
                    flush_pending()
                    if not (last and g == 1):
                        gw = 4096 if g == 0 else 4094
                        if last and g == 0:
                            # strip 3's g0 store rides the by-then-quiet SP
                            # ring, immediately (nothing queues behind it)
                            nc.sync.dma_start(
                                out=y[r0 : r0 + 126, 0:gw],
                                in_=stage[:126, :gw],
                            )
                        else:
                            pending.append((
                                y[r0 : r0 + 126, g * 4096 : g * 4096 + gw],
                                stage[:126, :gw],
                            ))

            for s in range(3):
                r0 = 126 * s
                if s == 0:
                    xt = xt0
                else:
                    xt = xpool.tile([128, WIDTH], F16, tag="xt")
                    chunks = [(0, 2048), (2048, 8192)] if s == 1 else [(0, 4096), (4096, 8192)]
                    for a, b in chunks:
                        nc.sync.dma_start(out=xt[:, a:b], in_=x[r0 : r0 + 128, a:b])
                full_strip(s, xt, last=False)

            # strip-3 loads first: xt3 reuses s0's buffer (free earliest)
            # so its 2MB lands early. The mini-strip tile follows (s1's
            # buffer); its im2col layout -- partition 3r + dj holds
            # X[504+r, dj:] -- means one K=30 matmul per col tile instead
            # of three K=10 ones, and the dj-interleaved partitions spread
            # each 10-row load over 8 SBUF ports instead of 2-3.
            xt3 = xpool.tile([128, WIDTH], F16, tag="xt")
            for a, b in [(0, 4096), (4096, 8192)]:
                nc.sync.dma_start(out=xt3[:, a:b], in_=x[378 : 378 + 128, a:b])
            x4 = xpool.tile([128, WIDTH], F16, tag="xt")
            for dj in range(KW):
                nc.sync.dma_start(
                    out=x4[dj : 28 + dj + 1 : 3, 0 : WIDTH - dj],
                    in_=x[504:514, dj:WIDTH],
                )

            full_strip(3, xt3, last=True, x4=x4)

    if split_waits:
        _split_multi_waits(nc)
    return nc


_NC_CACHE = None


def _get_nc():
    global _NC_CACHE
    if _NC_CACHE is None:
        _NC_CACHE = _build()
    return _NC_CACHE


def _make_host_inputs(X, W, b):
    Xh = np.ascontiguousarray(np.asarray(X, dtype=np.float32).astype(np.float16))
    W = np.asarray(W, dtype=np.float32)
    b = np.asarray(b, dtype=np.float32)

    bands = np.zeros((128, 3 * 128), dtype=np.float16)
    mm = np.arange(126)
    for dj in range(KW):
        for dk in range(KH):
            # B_dj[m+dk, m] = W[dk, dj] for every output row m
            bands[mm + dk, dj * 128 + mm] = W[dk, dj]
    # mini-strip im2col band: partition 3r + dj = input local row 504+r
    # shifted by dj cols; col m = output local row 504+m; B4[3r+dj, m] =
    # W[r-m, dj]
    bands4 = np.zeros((32, 8), dtype=np.float16)
    m8 = np.arange(8)
    for dj in range(KW):
        for dk in range(KH):
            bands4[3 * (m8 + dk) + dj, m8] = W[dk, dj]
    bias = np.full((128, 1), float(b[0]), dtype=np.float32)

    in_maps = []
    for i in range(N_CORES):
        r0 = i * RPC
        avail = min(IN_ROWS, H - r0)
        if avail == IN_ROWS:
            shard = Xh[r0 : r0 + IN_ROWS]
        else:
            shard = np.zeros((IN_ROWS, WIDTH), dtype=np.float16)
            shard[:avail] = Xh[r0 : r0 + avail]
        in_maps.append({"x": shard, "bands": bands, "bands4": bands4, "bias": bias})
    return in_maps


def _assemble(results):
    out = np.empty((OH, OW), dtype=np.float32)
    for i in range(N_CORES):
        r0 = i * RPC
        take = min(RPC, OH - r0)
        out[r0 : r0 + take] = results[i]["y"][:take].astype(np.float32)
    return out


def run(X, W, b, trace=False):
    nc = _get_nc()
    in_maps = _make_host_inputs(X, W, b)
    res = run_bass_kernel_spmd(nc, in_maps, list(range(N_CORES)), trace=trace)
    return _assemble(res.results), res


def kernel(X, W, b):
    out, _ = run(X, W, b)
    return out


# revision 17
# speedup vs baseline: 1.2291x; 1.0553x over previous
"""3x3 valid conv (single channel) on 8 TRN2 NeuronCores.

Strategy: fp16 end-to-end. The problem is memory-bound at fp32
(34MB/core); converting X to fp16 on host and storing y as fp16 (upcast
on host) halves HBM traffic to ~17MB/core. fp16 matmul runs at 1
row/cycle with exact f32 PSUM accumulation, so the only precision cost
is input/output rounding: ~8e-4 relative -- far inside the 2e-2 gate.
At fp16 the PE becomes the critical engine, so the schedule minimizes
PE moving rows and keeps the stream continuous.

Sharding: core c computes output rows 504c..504c+503 as four banded
strips of 126 rows (3 matmuls per 512-col PSUM tile; the banded
stationary folds the 3 vertical taps, the moving-column shifts fold the
horizontal ones -- within ~2% of the PE's information-theoretic floor
for a 3x3 conv). The 62 leftover rows (4032..4093) are redistributed
COLUMN-wise across all 8 cores as two host-prepared im2col strips
(M=40 and M=22, K=3*rows partitions, one matmul per 512-col unit), so
no core pays the old 8-row mini-strip's 8190 moving rows + 16
full-cost drains for 1.5% of the output. All cores run one identical
program; core 7's narrower edge is handled by host-side zero padding
and discard.

Startup: the first matmul needs the band + the first X chunk; each DMA
ring has ~2.5us of cold latency, so the band rides sync (earliest
ring) and the first two X chunks ride gpsimd/scalar in parallel.
Meanwhile the PE ramps its p-state (0.65->2.4GHz) on dummy matmuls
over a memset tile so the real stream starts at speed. PSUM drains
(bias add + f32->fp16 cast) alternate vector/scalar so a single
engine's drain rate can't gate the PE; drain cost is proportional to
free-dim length, so drains exist only per 512-col tile.
"""

import sys

sys.path.insert(0, "/opt/trn_rl_repo")

import numpy as np
from concourse import bass, mybir
from concourse.bass_utils import run_bass_kernel_spmd
from concourse.tile import TileContext

F32 = mybir.dt.float32
F16 = mybir.dt.float16

H, WIDTH = 4096, 8192
KH, KW = 3, 3
OH, OW = H - KH + 1, WIDTH - KW + 1
N_CORES = 8
RPC = 504                   # main output rows per core (4 strips of 126)
IN_ROWS = RPC + KH - 1      # 506 input rows per core
N_COL_TILES = 16            # 15 x 512 + 1 x 510 = 8190
RW = 1024                   # remainder column window per core
R_A, R_B = 40, 22           # remainder strip heights (rows 4032.. / 4072..)


def _split_multi_waits(nc, max_waits=1):
    # This container's walrus rejects >1 sync-wait command per instruction
    # (CoreV3 setupSyncWait). Tile attaches one wait per producing logical
    # processor to a single instruction; hoist the excess onto same-engine
    # Drain carriers inserted immediately before it.
    for fn in nc.m.functions:
        for bb in fn.blocks:
            out = []
            changed = False
            for inst in bb.instructions:
                si = inst.sync_info
                waits = list(si.on_wait) if si and si.on_wait else []
                if len(waits) > max_waits:
                    rest = waits[max_waits:]
                    for j in range(0, len(rest), max_waits):
                        carrier = mybir.InstDrain(
                            name=nc.get_next_instruction_name(), ins=[], outs=[]
                        )
                        carrier.engine = inst.engine
                        carrier.sync_info = mybir.SyncInfo(
                            on_wait=rest[j : j + max_waits], on_update=[]
                        )
                        out.append(carrier)
                    si.on_wait = waits[:max_waits]
                    changed = True
                out.append(inst)
            if changed:
                bb.instructions = out


def _build(split_waits=True):
    nc = bass.Bass()
    x = nc.declare_dram_parameter("x", [IN_ROWS, WIDTH], F16, isOutput=False)
    bands = nc.declare_dram_parameter("bands", [128, 3 * 128], F16, isOutput=False)
    banda = nc.declare_dram_parameter("banda", [3 * (R_A + 2), R_A], F16, isOutput=False)
    bandb = nc.declare_dram_parameter("bandb", [3 * (R_B + 2), R_B], F16, isOutput=False)
    xra = nc.declare_dram_parameter("xra", [3 * (R_A + 2), RW], F16, isOutput=False)
    xrb = nc.declare_dram_parameter("xrb", [3 * (R_B + 2), RW], F16, isOutput=False)
    bias = nc.declare_dram_parameter("bias", [128, 1], F32, isOutput=False)
    y = nc.declare_dram_parameter("y", [RPC, OW], F16, isOutput=True)
    yr = nc.declare_dram_parameter("yr", [R_A + R_B, RW], F16, isOutput=True)

    ident = mybir.ActivationFunctionType.Identity

    with TileContext(nc) as tc:
        with (
            tc.tile_pool(name="const", bufs=1) as cpool,
            tc.tile_pool(name="xin", bufs=3) as xpool,
            tc.tile_pool(name="stage", bufs=3) as spool,
            tc.tile_pool(name="psum", bufs=8, space="PSUM") as ppool,
        ):
            # --- PE p-state warmup: dummy matmuls over a memset tile.
            # No DMA dependency, so they start right after the preamble
            # barrier and the clock is ramping while the first X chunk is
            # still in flight.
            warm_t = cpool.tile([128, 512], F16)
            nc.gpsimd.memset(warm_t[:], 0.0)
            ps_w = ppool.tile([128, 512], F32, tag="ps")
            for _ in range(4):
                nc.tensor.matmul(
                    ps_w[:126, :512],
                    warm_t[:, 0:126],
                    warm_t[:, 0:512],
                    start=True,
                    stop=True,
                )

            # --- constants and first strip chunks, ordered so each of the
            # three cold DMA rings starts its first transfer immediately:
            # sync gets the band (the first matmul's LDWEIGHTS needs it),
            # gpsimd and scalar each get one early X chunk. The tiny bias
            # and remainder bands queue behind gpsimd's chunk.
            band_f = cpool.tile([128, 3 * 128], F16)
            nc.sync.dma_start(out=band_f[:], in_=bands[:])
            xt0 = xpool.tile([128, WIDTH], F16, tag="xt")
            nc.gpsimd.dma_start(out=xt0[:, 0:528], in_=x[0:128, 0:528])
            nc.scalar.dma_start(out=xt0[:, 528:1040], in_=x[0:128, 528:1040])
            for a, b in [(1040, 1552), (1552, 2576), (2576, 4624), (4624, 8192)]:
                nc.sync.dma_start(out=xt0[:, a:b], in_=x[0:128, a:b])
            bias_t = cpool.tile([128, 1], F32)
            nc.gpsimd.dma_start(out=bias_t[:], in_=bias[:])
            banda_f = cpool.tile([3 * (R_A + 2), R_A], F16)
            nc.gpsimd.dma_start(out=banda_f[:], in_=banda[:])
            bandb_f = cpool.tile([3 * (R_B + 2), R_B], F16)
            nc.gpsimd.dma_start(out=bandb_f[:], in_=bandb[:])
            stage_r = cpool.tile([128, RW], F16)
            # prime the ACT function table (1.3us, once) before the first
            # scalar drain needs it
            prime_t = cpool.tile([128, 1], F32)
            nc.scalar.activation(prime_t[:1, :], bias_t[:1, :], ident,
                                 bias=bias_t[:1, :], scale=1.0)

            def drain(ct, dst, src, npart, base=0):
                # alternate engines so neither gates the PE
                bt = bias_t[base : base + npart, :]
                if ct % 2 == 0:
                    nc.scalar.activation(dst, src, ident, bias=bt, scale=1.0)
                else:
                    nc.vector.tensor_scalar_add(dst, src, bt)

            # scalar store-issues are LAGGED by one group: emitted only
            # after the next group's drains, so their drain-sems are already
            # satisfied and the scalar queue (which also carries half the
            # drains) never head-of-line blocks
            pending = []

            def flush_pending():
                while pending:
                    dst, srcap = pending.pop(0)
                    nc.scalar.dma_start(out=dst, in_=srcap)

            def remainder(xra_f, xrb_f):
                # the shared 62 leftover rows, this core's 1024-col window:
                # two im2col strips (M=40 at stage partitions 0.., M=22 at
                # 64.. -- engine partition bases must be 0/32/64), one
                # matmul per 512-col unit, one drain each, one store pair.
                for u in range(2):
                    c0 = 512 * u
                    psa = ppool.tile([128, 512], F32, tag="ps")
                    nc.tensor.matmul(
                        psa[:R_A, :512],
                        banda_f[:, :R_A],
                        xra_f[:, c0 : c0 + 512],
                        start=True,
                        stop=True,
                    )
                    drain(u, stage_r[:R_A, c0 : c0 + 512], psa[:R_A, :512], R_A)
                    psb = ppool.tile([128, 512], F32, tag="ps")
                    nc.tensor.matmul(
                        psb[:R_B, :512],
                        bandb_f[:, :R_B],
                        xrb_f[:, c0 : c0 + 512],
                        start=True,
                        stop=True,
                    )
                    drain(u + 1, stage_r[64 : 64 + R_B, c0 : c0 + 512],
                          psb[:R_B, :512], R_B, base=64)
                nc.sync.dma_start(out=yr[0:R_A, :], in_=stage_r[0:R_A, :])
                nc.sync.dma_start(
                    out=yr[R_A : R_A + R_B, :], in_=stage_r[64 : 64 + R_B, :]
                )

            def full_strip(s, xt, last):
                r0 = 126 * s
                for g in range(2):
                    stage = spool.tile([128, 4096], F16, tag="st")
                    for j in range(8):
                        ct = g * 8 + j
                        c0 = ct * 512
                        n = 512 if ct < N_COL_TILES - 1 else 510
                        ps = ppool.tile([128, 512], F32, tag="ps")
                        for dj in range(KW):
                            nc.tensor.matmul(
                                ps[:126, :n],
                                band_f[:, dj * 128 : dj * 128 + 126],
                                xt[:, c0 + dj : c0 + dj + n],
                                start=(dj == 0),
                                stop=(dj == KW - 1),
                            )
                        drain(ct, stage[:126, j * 512 : j * 512 + n],
                              ps[:126, :n], 126)
                        if last and g == 1 and j in (2, 4, 5, 6, 7):
                            # drip out the final stage in pieces so the very
                            # last store is small
                            lo = {2: 0, 4: 1024, 5: 2048, 6: 3072, 7: 3584}[j]
                            hi = {2: 1024, 4: 2048, 5: 3072, 6: 3584, 7: 4094}[j]
                            eng = nc.sync if j == 7 else nc.scalar
                            eng.dma_start(
                                out=y[r0 : r0 + 126, 4096 + lo : 4096 + hi],
                                in_=stage[:126, lo:hi],
                            )
                    flush_pending()
                    if not (last and g == 1):
                        gw = 4096 if g == 0 else 4094
                        if last and g == 0:
                            # strip 3's g0 store rides the by-then-quiet SP
                            # ring, immediately (nothing queues behind it)
                            nc.sync.dma_start(
                                out=y[r0 : r0 + 126, 0:gw],
                                in_=stage[:126, :gw],
                            )
                        else:
                            pending.append((
                                y[r0 : r0 + 126, g * 4096 : g * 4096 + gw],
                                stage[:126, :gw],
                            ))

            xra_f = cpool.tile([3 * (R_A + 2), RW], F16)
            xrb_f = cpool.tile([3 * (R_B + 2), RW], F16)
            for s in range(3):
                r0 = 126 * s
                if s == 0:
                    xt = xt0
                else:
                    xt = xpool.tile([128, WIDTH], F16, tag="xt")
                    chunks = [(0, 2048), (2048, 8192)] if s == 1 else [(0, 4096), (4096, 8192)]
                    for a, b in chunks:
                        nc.sync.dma_start(out=xt[:, a:b], in_=x[r0 : r0 + 128, a:b])
                if s == 1:
                    # remainder inputs (405KB) land behind strip 1's chunks
                    nc.sync.dma_start(out=xra_f[:], in_=xra[:])
                    nc.sync.dma_start(out=xrb_f[:], in_=xrb[:])
                full_strip(s, xt, last=False)
                if s == 1:
                    # remainder compute rides between strips 1 and 2: a
                    # 0.9us PE block whose drains/stores vanish in the
                    # mid-run slack
                    remainder(xra_f, xrb_f)

            # strip-3 loads reuse s0's buffer (free earliest); they land
            # well before ~40us
            xt3 = xpool.tile([128, WIDTH], F16, tag="xt")
            for a, b in [(0, 4096), (4096, 8192)]:
                nc.sync.dma_start(out=xt3[:, a:b], in_=x[378 : 378 + 128, a:b])

            full_strip(3, xt3, last=True)

    if split_waits:
        _split_multi_waits(nc)
    return nc


_NC_CACHE = None


def _get_nc():
    global _NC_CACHE
    if _NC_CACHE is None:
        _NC_CACHE = _build()
    return _NC_CACHE


def _make_host_inputs(X, W, b):
    Xh = np.ascontiguousarray(np.asarray(X, dtype=np.float32).astype(np.float16))
    W = np.asarray(W, dtype=np.float32)
    b = np.asarray(b, dtype=np.float32)

    bands = np.zeros((128, 3 * 128), dtype=np.float16)
    mm = np.arange(126)
    for dj in range(KW):
        for dk in range(KH):
            # B_dj[m+dk, m] = W[dk, dj] for every output row m
            bands[mm + dk, dj * 128 + mm] = W[dk, dj]

    def im2col_band(rows):
        # B[3(m+dk)+dj, m] = W[dk, dj]: partition 3r+dj holds input row
        # base+r shifted dj cols; output row m uses input rows m..m+2
        bnd = np.zeros((3 * (rows + 2), rows), dtype=np.float16)
        m = np.arange(rows)
        for dj in range(KW):
            for dk in range(KH):
                bnd[3 * (m + dk) + dj, m] = W[dk, dj]
        return bnd

    banda = im2col_band(R_A)
    bandb = im2col_band(R_B)
    bias = np.full((128, 1), float(b[0]), dtype=np.float32)

    def im2col_x(base_row, rows, w0):
        # xr[3r+dj, j] = X[base_row+r, w0+dj+j], zero past the right edge
        xr = np.zeros((3 * (rows + 2), RW), dtype=np.float16)
        for r in range(rows + 2):
            for dj in range(KW):
                c0 = w0 + dj
                c1 = min(c0 + RW, WIDTH)
                if c1 > c0:
                    xr[3 * r + dj, : c1 - c0] = Xh[base_row + r, c0:c1]
        return xr

    in_maps = []
    for i in range(N_CORES):
        r0 = i * RPC
        shard = Xh[r0 : r0 + IN_ROWS]
        w0 = i * RW
        in_maps.append({
            "x": shard,
            "bands": bands,
            "banda": banda,
            "bandb": bandb,
            "xra": im2col_x(4032, R_A, w0),
            "xrb": im2col_x(4072, R_B, w0),
            "bias": bias,
        })
    return in_maps


def _assemble(results):
    out = np.empty((OH, OW), dtype=np.float32)
    for i in range(N_CORES):
        r0 = i * RPC
        out[r0 : r0 + RPC] = results[i]["y"].astype(np.float32)
        w0 = i * RW
        w = min(RW, OW - w0)
        out[4032 : 4032 + R_A + R_B, w0 : w0 + w] = (
            results[i]["yr"][:, :w].astype(np.float32)
        )
    return out


def run(X, W, b, trace=False):
    nc = _get_nc()
    in_maps = _make_host_inputs(X, W, b)
    res = run_bass_kernel_spmd(nc, in_maps, list(range(N_CORES)), trace=trace)
    return _assemble(res.results), res


def kernel(X, W, b):
    out, _ = run(X, W, b)
    return out
